# revision 46
# baseline (speedup 1.0000x reference)
"""Trainium2 Bass kernel for per-token head-mixing attention.

Reference computation (per token s):
    q,k,v = x @ W{q,k,v}.T + b{q,k,v}          (HIDDEN=1024 -> 16 heads x 64)
    energy[s,k,m] = (q[s,k,:] . k[s,m,:]) / 8
    attn = softmax_m(energy);  out[s,k,:] = sum_m attn[s,k,m] * v[s,m,:]

Strategy (8 NeuronCores, data-parallel over the 16384 tokens, 2048/core):
  * token-row layout [128 tokens (partitions), features] everywhere
  * PE: QKV projections (lhsT = x^T chunks, rhs = W^T chunks; bias via an
    appended ones-row in x^T / bias-row in W^T), and attn@v as block-diagonal
    matmuls packing 8 tokens per matmul (contraction = (token,m_head)).
  * DVE: energy pair-products via broadcast-AP tensor_mul (bf16 2x mode) in
    (q-half, k-half) quarters that start as soon as the first k/q PSUM
    evictions land, then tree-add reduce over head_dim (also bf16 2x),
    softmax normalize via scalar_tensor_tensor. DVE is the critical engine;
    the kernel runs at ~its roofline (products+tree are ~180us of the
    ~200-220us/forward measured on HW).
  * ACT: exp (with the 1/sqrt(hd) scale folded in) and PSUM->SBUF evictions
    ordered k0,q0,k1,q1,v so energy work starts earliest.
  * DMA: block-diagonal attn tiles and stacked-v tiles are gathered through
    small HBM staging buffers with rectangular strided access patterns;
    initial x/weight loads are spread across 3 DMA queues in first-use
    order. The bd tiles are persistent with one-time memsets (the gathers
    only ever rewrite the same diagonal blocks, so the zeros survive).
"""

import os

import numpy as np

HIDDEN = 1024
NH = 16
HD = 64
B = 4
SEQ = 4096
NCORES = 8
S_CORE = (B * SEQ) // NCORES  # tokens per core

_PROGRAM_CACHE = {}


def build_program(S, dt_name="bfloat16", gp_l1q=0, lag=2, loop_reps=1, stage="full", q_split=2, prod_bufs=3, loop_unroll=1):
    """Build the (SPMD, per-core) Bass program for S tokens.

    gp_l1q: per tile, the first gp_l1q energy quarters (0..4) run their
    (largest) level-1 tree add on GPSIMD instead of DVE, offloading the
    critical DVE engine without long GPSIMD chains on the critical path.
    lag: software pipeline depth (attn phase lags qkv phase by `lag` tiles).
    loop_reps: >1 wraps the whole per-tile loop (including the x input
    loads, excluding the one-time weight loads) in a tc.For_i hardware
    loop, re-running the full computation loop_reps times in one dispatch.
    Used by the timing harness: the wall-clock difference between the
    loop_reps=K and loop_reps=1 programs isolates on-device execution time
    from the ~100 ms axon RPC dispatch overhead.
    """
    from contextlib import ExitStack

    import concourse.bass as bass
    import concourse.tile as tile
    from concourse import bacc, mybir
    from bass_rust import add_dep_helper

    dt_c = getattr(mybir.dt, dt_name)
    f32 = mybir.dt.float32
    NT = S // 128
    KC = HIDDEN // 128  # contraction chunks

    nc = bacc.Bacc()
    xT = nc.declare_dram_parameter("xT", [HIDDEN + 1, S], dt_c, isOutput=False)
    wq = nc.declare_dram_parameter("wq", [HIDDEN + 1, HIDDEN], dt_c, isOutput=False)
    wk = nc.declare_dram_parameter("wk", [HIDDEN + 1, HIDDEN], dt_c, isOutput=False)
    wv = nc.declare_dram_parameter("wv", [HIDDEN + 1, HIDDEN], dt_c, isOutput=False)
    out = nc.declare_dram_parameter("out", [S, HIDDEN], f32, isOutput=True)
    v_stage = nc.dram_tensor("v_stage", [S, HIDDEN], dt_c)
    a_stage = nc.dram_tensor("a_stage", [S, NH * NH], dt_c)

    with tile.TileContext(nc) as tc, ExitStack() as ctx:
        singles = ctx.enter_context(tc.tile_pool(name="singles", bufs=1))
        qkv_psum = ctx.enter_context(tc.tile_pool(name="qkvps", bufs=6, space="PSUM"))
        av_psum = ctx.enter_context(tc.tile_pool(name="avps", bufs=2, space="PSUM"))
        qkvt = ctx.enter_context(tc.tile_pool(name="qkvt", bufs=lag + 2))
        out_pool = ctx.enter_context(tc.tile_pool(name="outp", bufs=2))
        prod_pool = ctx.enter_context(tc.tile_pool(name="prod", bufs=prod_bufs))
        small = ctx.enter_context(tc.tile_pool(name="small", bufs=3))
        v8_pool = ctx.enter_context(tc.tile_pool(name="v8", bufs=2))

        # Resident weights (8 chunk tiles + 1 bias row per projection) and
        # x^T (8 chunk tiles [128, S] + ones row), loaded once. Loads are
        # interleaved in first-use order (xT/wk/wq chunk-by-chunk, wv last)
        # and spread across DMA queues so tile 0's QKV matmuls can start as
        # soon as the first (xT, w) chunk pair lands instead of after the
        # whole 7 MB of parameters.
        w_sb = {}
        chunk_tiles = {}
        for name, w in (("q", wq), ("k", wk), ("v", wv)):
            chunks = []
            for kc in range(KC):
                t = singles.tile([128, HIDDEN], dt_c, tag=f"w{name}{kc}")
                chunks.append(t)
            tb = singles.tile([1, HIDDEN], dt_c, tag=f"w{name}b")
            w_sb[name] = (chunks, tb)
            chunk_tiles[name] = (chunks, tb, w)
        xts = []
        for kc in range(KC):
            t = singles.tile([128, S], dt_c, tag=f"xt{kc}")
            xts.append(t)
        xtb = singles.tile([1, S], dt_c, tag="xtb")

        load_engines = [nc.sync, nc.scalar, nc.gpsimd]

        def load(dst, src, q):
            load_engines[q % len(load_engines)].dma_start(out=dst, in_=src)

        def load_x(kc):
            load(xts[kc], xT[kc * 128 : (kc + 1) * 128, :], kc)

        def load_w(name, kc, q):
            chunks, tb, w = chunk_tiles[name]
            load(chunks[kc], w[kc * 128 : (kc + 1) * 128, :], q)

        for kc in range(KC):
            if loop_reps == 1:
                load_x(kc)
            load_w("k", kc, kc + 1)
            load_w("q", kc, kc + 2)
        if loop_reps == 1:
            load(xtb, xT[HIDDEN : HIDDEN + 1, :], 0)
        for j, name in enumerate(("k", "q", "v")):
            chunks, tb, w = chunk_tiles[name]
            if name == "v":
                for kc in range(KC):
                    load_w("v", kc, kc)
            load(tb, w[HIDDEN : HIDDEN + 1, :], j)

        # Persistent block-diagonal attn tiles, explicit 3-way round-robin.
        # The gathers only ever write the same diagonal-block pattern, so the
        # off-diagonal zeros established by these one-time memsets (GPSIMD,
        # which is otherwise idle at program start) persist for all tiles.
        bd_tiles = []
        for i in range(3):
            t = singles.tile([128, 16 * 128], dt_c, tag=f"bd{i}")
            nc.gpsimd.memset(t.bitcast(mybir.dt.int32), 0)
            bd_tiles.append(t)

        fence_protected = []  # (gather_dma, staging_write) pairs

        def qkv_phase(it):
            tok0 = it * 128
            xcs = [t[:, tok0 : tok0 + 128] for t in xts]
            xb = xtb[:, tok0 : tok0 + 128]
            sb = {}
            for name in ("q", "k", "v"):
                t_proj = qkvt.tile([128, HIDDEN], dt_c, tag=f"t{name}")
                sb[name] = t_proj
            # (proj, half) emission order chosen so the energy quarter
            # products (which need k-half m and q-half g) can start as early
            # as possible: k0, q0, k1, q1, then v.
            for name, h in (
                ("k", 0),
                ("q", 0),
                ("k", 1),
                ("q", 1),
                ("v", 0),
                ("v", 1),
            ):
                chunks, bias_row = w_sb[name]
                t_sb = sb[name]
                ps = qkv_psum.tile([128, 512], f32, tag="ps")
                for kc in range(KC):
                    nc.tensor.matmul(
                        ps,
                        xcs[kc],
                        chunks[kc][:, h * 512 : (h + 1) * 512],
                        start=(kc == 0),
                        stop=False,
                    )
                nc.tensor.matmul(
                    ps,
                    xb,
                    bias_row[:, h * 512 : (h + 1) * 512],
                    start=False,
                    stop=True,
                )
                nc.scalar.copy(t_sb[:, h * 512 : (h + 1) * 512], ps)
            return sb

        def attn_phase(it, sb):
            tok0 = it * 128
            if stage == "qkv":
                # timing-only variant: stop after the projections; dump v
                # to the output region so the program still writes `out`.
                nc.scalar.dma_start(
                    out=out[tok0 : tok0 + 128, :].bitcast(dt_c)[:, 0:HIDDEN],
                    in_=sb["v"],
                )
                return
            if stage != "noav":
                # ---- stage v (token-major) to HBM for the stacked-v gather
                v_wr = nc.scalar.dma_start(
                    out=v_stage[tok0 : tok0 + 128, :], in_=sb["v"]
                )

            # ---- energy: pair products (DVE bf16 2x) + grouped tree reduce
            # over d, in (q-half g, k-piece m) pieces so work starts as soon
            # as the first k/q evictions land. A tunable number of pieces'
            # (largest) level-1 tree adds can go to GPSIMD (gp_l1q), though
            # on real HW GPSIMD adds are slow and gp_l1q=0 wins.
            energy = small.tile([128, NH * NH], f32, tag="energy")
            ev = energy.rearrange("p (a b) -> p a b", b=NH)
            qv = sb["q"].rearrange("p (k d) -> p k d", d=HD)
            kv = sb["k"].rearrange("p (m d) -> p m d", d=HD)
            ms = NH // (q_split // 2)  # k-heads per piece
            pieces = [(g, m) for g in range(2) for m in range(q_split // 2)]
            for piece, (g, m) in enumerate(pieces):
                prod = prod_pool.tile([128, 8, ms, HD], dt_c, tag="prod")
                in0 = (
                    qv[:, g * 8 : (g + 1) * 8, :]
                    .unsqueeze(2)
                    .broadcast_to((128, 8, ms, HD))
                )
                in1 = (
                    kv[:, m * ms : (m + 1) * ms, :]
                    .unsqueeze(1)
                    .broadcast_to((128, 8, ms, HD))
                )
                nc.vector.tensor_mul(prod, in0, in1)
                # tree-add reduce over d (bf16 TT runs at 2x on DVE)
                pv = prod.rearrange("p a b d -> p (a b) d")
                w = HD
                level = 0
                while w > 2:
                    tree_eng = (
                        nc.gpsimd
                        if (level == 0 and piece < gp_l1q)
                        else nc.vector
                    )
                    tree_eng.tensor_add(
                        pv[:, :, 0 : w // 2],
                        pv[:, :, 0 : w // 2],
                        pv[:, :, w // 2 : w],
                    )
                    w //= 2
                    level += 1
                nc.vector.tensor_add(
                    ev[:, g * 8 : (g + 1) * 8, m * ms : (m + 1) * ms],
                    prod[:, :, :, 0],
                    prod[:, :, :, 1],
                )

            # ---- softmax over m: exp (scale folds the 1/sqrt(hd))
            expt = small.tile([128, NH * NH], f32, tag="expt")
            nc.scalar.activation(
                expt, energy, mybir.ActivationFunctionType.Exp, scale=0.125
            )
            # dsum on DVE (tiny; keeping it off GPSIMD avoids queueing the
            # softmax chain behind 6us quarter-pools)
            dsum = small.tile([128, NH], f32, tag="dsum")
            nc.vector.tensor_reduce(
                out=dsum,
                in_=expt.rearrange("p (k m) -> p k m", m=NH),
                axis=mybir.AxisListType.X,
                op=mybir.AluOpType.add,
            )
            rec = small.tile([128, NH], f32, tag="rec")
            nc.vector.reciprocal(rec, dsum)
            # attn = expt * rec, written (m-major, k-minor), bf16
            attn = small.tile([128, NH * NH], dt_c, tag="attn")
            nc.vector.scalar_tensor_tensor(
                out=attn.rearrange("p (m k) -> p k m", k=NH),
                in0=expt.rearrange("p (k m) -> p k m", m=NH),
                scalar=1.0,
                in1=rec.unsqueeze(2).broadcast_to((128, NH, NH)),
                op0=mybir.AluOpType.mult,
                op1=mybir.AluOpType.mult,
            )
            if stage == "noav":
                # timing-only variant: stop after softmax; dump attn to the
                # output region so the program still writes `out`.
                nc.scalar.dma_start(
                    out=out[tok0 : tok0 + 128, :].bitcast(dt_c)[:, 0 : NH * NH],
                    in_=attn,
                )
                return
            a_wr = nc.scalar.dma_start(out=a_stage[tok0 : tok0 + 128, :], in_=attn)

            # ---- gather block-diagonal attn tiles: 16 blocks of [16m x 16k]
            # bd partition p = 16*t' + m ; column = 128*T + 16*t' + k
            bd = bd_tiles[it % 3]
            for tp in range(8):
                src = bass.AP(
                    tensor=a_stage,
                    offset=(tok0 + tp) * (NH * NH),
                    ap=[[NH, NH], [8 * NH * NH, 16], [1, NH]],  # (m, T, k)
                )
                dst = bd[tp * 16 : (tp + 1) * 16, :].rearrange(
                    "p (T x) -> p T x", x=128
                )[:, :, tp * 16 : tp * 16 + 16]
                eng = nc.sync if tp % 2 == 0 else nc.scalar
                rd = eng.dma_start(out=dst, in_=src)
                add_dep_helper(rd.ins, a_wr.ins, sync=True, reason="a_stage RAW")
                fence_protected.append((rd, a_wr))

            # ---- gather stacked v: v8 partition p = 16*t' + m, col = 64*T + d
            v8 = v8_pool.tile([128, 16 * HD], dt_c, tag="v8")
            src = bass.AP(
                tensor=v_stage,
                offset=tok0 * HIDDEN,
                ap=[[HD, 128], [8 * HIDDEN, 16], [1, HD]],  # (p, T, d)
            )
            v_rd = nc.sync.dma_start(
                out=v8.rearrange("p (T d) -> p T d", d=HD), in_=src
            )
            add_dep_helper(v_rd.ins, v_wr.ins, sync=True, reason="v_stage RAW")
            fence_protected.append((v_rd, v_wr))

            # ---- attn@v: one matmul per 8-token group
            out_t = out_pool.tile([128, 16 * HD], f32, tag="out")
            for half in range(2):
                ops = av_psum.tile([128, 8 * HD], f32, tag="avps")
                for Tg in range(8):
                    T = half * 8 + Tg
                    nc.tensor.matmul(
                        ops[:, Tg * HD : (Tg + 1) * HD],
                        bd[:, T * 128 : (T + 1) * 128],
                        v8[:, T * HD : (T + 1) * HD],
                        start=True,
                        stop=True,
                    )
                nc.scalar.copy(
                    out_t[:, half * 8 * HD : (half + 1) * 8 * HD], ops
                )
            # out[s, k*64+d], s = tok0 + 8*T + t', psum p = 16*t' + k
            dst = bass.AP(
                tensor=out,
                offset=tok0 * HIDDEN,
                ap=[[HD, 128], [8 * HIDDEN, 16], [1, HD]],  # (p, T, d)
            )
            nc.scalar.dma_start(
                out=dst, in_=out_t.rearrange("p (T d) -> p T d", d=HD)
            )

        # `lag`-stage software pipeline: attn(it) is emitted after
        # qkv(it+lag-1), so each ~20us attn chain overlaps several dense PE
        # bursts and the scheduler can keep every engine fed.
        def main_body():
            if loop_reps > 1:
                # x loads live inside the timing loop (the per-rep work
                # includes re-reading the input from HBM; weights stay
                # resident as they would in steady-state serving)
                for kc in range(KC):
                    load_x(kc)
                load(xtb, xT[HIDDEN : HIDDEN + 1, :], 0)
            pending = []
            for it in range(NT + lag):
                if it < NT:
                    pending.append((it, qkv_phase(it)))
                if it >= lag:
                    pit, psb = pending.pop(0)
                    attn_phase(pit, psb)

        if loop_reps > 1:
            with tc.For_i(0, loop_reps):
                for _ in range(loop_unroll):
                    main_body()
        else:
            main_body()

    # Post-pass: a staged gather's cross-tile WAW waits on other DMA lanes
    # are implied by queue FIFO order / the engine (PE) WAR wait (Tile's sem
    # assignment is not transitively minimal and the DMA instruction only has
    # 2 hardware wait slots). Keep only the RAW wait on its own tile's
    # staging write plus any engine waits.
    for rd, wr in fence_protected:
        upd = wr.ins.sync_info.on_update
        keep_lanes = {u.ant_name for u in upd}
        si = rd.ins.sync_info
        new_waits = [
            w
            for w in si.on_wait
            if (not w.ant_name.startswith("DMAHW")) or w.ant_name in keep_lanes
        ]
        if len(new_waits) > 2:
            # A whole-tile fence (POOL memset at start, or DVE in older
            # variants) covers previous readers; prefer it over PE waits.
            has_fence = any(
                w.ant_name.startswith(("DVE", "POOL")) for w in new_waits
            )
            if has_fence:
                new_waits = [
                    w
                    for w in new_waits
                    if w.ant_name.startswith(("DVE", "POOL"))
                    or w.ant_name in keep_lanes
                ]
        if len(new_waits) != len(si.on_wait):
            si.on_wait = new_waits
            rd.ins.sync_info = si
    nwmax = 0
    for bb in nc.m.functions[0].blocks:
        for inst in bb.instructions:
            if type(inst).__name__ == "InstDMACopy":
                nwmax = max(nwmax, len(inst.sync_info.on_wait))
    nc.compile()
    if nwmax > 2:
        for bb in nc.m.functions[0].blocks:
            for inst in bb.instructions:
                if (
                    type(inst).__name__ == "InstDMACopy"
                    and len(inst.sync_info.on_wait) > 2
                ):
                    ln = inst.debug.lineno if inst.debug else "?"
                    print(
                        f"WARN {inst.name} line {ln}:",
                        [w.ant_name for w in inst.sync_info.on_wait],
                    )
    return nc


def _get_program(S, dt_name, **kw):
    key = (S, dt_name, tuple(sorted(kw.items())))
    if key not in _PROGRAM_CACHE:
        _PROGRAM_CACHE[key] = build_program(S, dt_name, **kw)
    return _PROGRAM_CACHE[key]


def _prep_inputs(x, Wq, bq, Wk, bk, Wv, bv, dt_np, S, ncores):
    """Host-side prep: transpose/shard/append bias rows, cast."""
    x2 = np.ascontiguousarray(np.asarray(x, np.float32).reshape(-1, HIDDEN))

    def prep_w(W, b):
        return np.ascontiguousarray(
            np.vstack(
                [np.asarray(W, np.float32).T, np.asarray(b, np.float32)[None, :]]
            )
        ).astype(dt_np)

    wqh = prep_w(Wq, bq)
    wkh = prep_w(Wk, bk)
    wvh = prep_w(Wv, bv)
    in_maps = []
    for c in range(ncores):
        xs = x2[c * S : (c + 1) * S].T  # [HIDDEN, S]
        xTc = np.vstack([xs, np.ones((1, S), np.float32)]).astype(dt_np)
        in_maps.append(
            {
                "xT": np.ascontiguousarray(xTc),
                "wq": wqh,
                "wk": wkh,
                "wv": wvh,
            }
        )
    return in_maps


LAST_RESULTS = None  # BassKernelResults of the most recent kernel() call


def kernel(x, Wq, bq, Wk, bk, Wv, bv):
    global LAST_RESULTS
    import ml_dtypes

    from concourse.bass_utils import run_bass_kernel_spmd

    dt_name = os.environ.get("KERNEL_DTYPE", "bfloat16")
    dt_np = (
        np.dtype(ml_dtypes.bfloat16) if dt_name == "bfloat16" else np.float32
    )
    trace = os.environ.get("KERNEL_TRACE", "0") == "1"
    nc = _get_program(S_CORE, dt_name)
    in_maps = _prep_inputs(x, Wq, bq, Wk, bk, Wv, bv, dt_np, S_CORE, NCORES)
    res = run_bass_kernel_spmd(nc, in_maps, list(range(NCORES)), trace=trace)
    LAST_RESULTS = res
    outs = [res.results[c]["out"] for c in range(NCORES)]
    return np.concatenate(outs, axis=0).reshape(B, SEQ, HIDDEN)

